# revision 10
# baseline (speedup 1.0000x reference)
# Multi-head attention (B=2, N=2048, C=1024, H=16) on 8 trn2 NeuronCores.
#
# Sharding: core = (batch b = core//4, head-group hg = core%4, 4 heads each).
# Each core computes qkv/attention/proj for its 4 heads of its batch and
# returns a partial projection output [N, C]; the host sums the 4 partials
# per batch and adds b_proj.
#
# Per-core device pipeline (all matmuls in float32r, full-rate at N>=256):
#   1. x [N,C] -> PE-transpose -> xT [C,N]                 (fp32 transposes)
#   2. qkvT[768, N] = Wsel @ x^T   (lhsT=wqkvT, rhs=xT)    -> q^T,k^T,v^T
#   3. v' = [v | 1] natural layout via PE-transpose of v^T
#   4. per (head) unit: S^T[j,i] = k @ q^T ; E=exp(S*scale) on ACT;
#      O'^T[65, N] += v'^T @ E^T  (row 64 = softmax denominator)
#      normalize with 1/rowsum broadcast (gpsimd partition_broadcast)
#   5. proj partial: out[i,e] = sum_ch O^T[ch,i] * wprojT[ch,e]
import sys

import numpy as np

if "/opt/trn_rl_repo" not in sys.path:
    sys.path.insert(0, "/opt/trn_rl_repo")

B, NSEQ, C = 2, 2048, 1024
H, HD = 16, 64
P = 128
SCALE = HD**-0.5

_cache = {}


def _build(nseq):
    from contextlib import ExitStack

    import concourse.tile as tile
    from concourse import bacc, mybir
    from concourse.masks import make_identity

    f32 = mybir.dt.float32
    f32r = mybir.dt.float32r
    EXP = mybir.ActivationFunctionType.Exp

    NIT = nseq // P          # i tiles (output rows / queries)
    NJT = nseq // P          # j tiles (keys)
    QCH = min(512, nseq)     # matmul moving-dim chunk
    SW = min(1024, nseq)     # S^T psum tile width (2 banks)
    NSW = nseq // SW
    NOB = nseq // QCH        # number of O' psum tiles
    ECH = 512                # proj output chunk

    nc = bacc.Bacc("TRN2", target_bir_lowering=False, debug=False, num_devices=8)
    x_d = nc.dram_tensor("x", [nseq, C], f32r, kind="ExternalInput")
    wq_d = nc.dram_tensor("wqkvT", [C, 6 * P], f32r, kind="ExternalInput")
    wp_d = nc.dram_tensor("wprojT", [P, 2, C], f32r, kind="ExternalInput")
    out_d = nc.dram_tensor("out", [nseq, C], f32, kind="ExternalOutput")

    cp_state = [0]

    def cp(out, in_):
        # alternate PSUM->SBUF copies between DVE and ACT
        cp_state[0] ^= 1
        if cp_state[0]:
            nc.vector.tensor_copy(out, in_)
        else:
            nc.scalar.copy(out, in_)

    with tile.TileContext(nc) as tc, ExitStack() as ctx:
        persist = ctx.enter_context(tc.tile_pool(name="persist", bufs=1))
        qkT = persist.tile([P, 4, nseq], f32r)
        ones_f32 = persist.tile([P, 1], f32)
        nc.vector.memset(ones_f32, 1.0)
        v1 = persist.tile([P, 4, NJT, HD + 1], f32r)

        # ======== scope A: phases 1-3 ========
        with (
            tc.tile_pool(name="scopeA", bufs=1) as scopeA,
            tc.tile_pool(name="xin", bufs=3) as xin,
            tc.tile_pool(name="psA", bufs=3, space="PSUM") as psA,
            tc.tile_pool(name="psAv", bufs=2, space="PSUM") as psAv,
            tc.tile_pool(name="psQ", bufs=2, space="PSUM") as psQ,
        ):
            ident = scopeA.tile([P, P], f32)
            make_identity(nc, ident)
            identR = scopeA.tile([P, P], f32r)
            nc.vector.tensor_copy(identR, ident)
            xT = scopeA.tile([P, 8, nseq], f32r)
            vT = scopeA.tile([P, 2, nseq], f32)

            wq_sb = scopeA.tile([P, 8, 6 * P], f32r)
            nc.gpsimd.dma_start(wq_sb, wq_d.ap().rearrange("(co p) d -> p co d", p=P))
            wp_sb = persist.tile([P, 2, C], f32r)
            nc.gpsimd.dma_start(wp_sb, wp_d.ap())

            # ---- Phase 1: transpose x into xT[c_part, c_outer, i] ----
            for it in range(NIT):
                xt = xin.tile([P, C], f32r)
                nc.sync.dma_start(xt, x_d[it * P : (it + 1) * P, :])
                for cg in range(2):
                    ps = psA.tile([P, 4, P], f32r, tag="psA")
                    for k in range(4):
                        cch = cg * 4 + k
                        nc.tensor.transpose(
                            ps[:, k, :], xt[:, cch * P : (cch + 1) * P], identR
                        )
                    cp(xT[:, cg * 4 : cg * 4 + 4, it * P : (it + 1) * P], ps)

            # ---- Phase 2: qkvT[p, mt, i] ----
            for mt in range(6):
                for nch in range(nseq // QCH):
                    ps = psQ.tile([P, QCH], f32, tag="psQ")
                    for co in range(8):
                        nc.tensor.matmul(
                            ps,
                            lhsT=wq_sb[:, co, mt * P : (mt + 1) * P],
                            rhs=xT[:, co, nch * QCH : (nch + 1) * QCH],
                            start=(co == 0),
                            stop=(co == 7),
                        )
                    dest = (
                        qkT[:, mt, nch * QCH : (nch + 1) * QCH]
                        if mt < 4
                        else vT[:, mt - 4, nch * QCH : (nch + 1) * QCH]
                    )
                    cp(dest, ps)

            # ---- Phase 3: v' natural [j_part, u, jt, 65] with ones column ----
            nc.vector.tensor_copy(
                v1[:, :, :, HD : HD + 1],
                ones_f32[:, None, None, :].to_broadcast([P, 4, NJT, 1]),
            )
            VB = min(4, NJT)
            for u in range(4):
                pb = 64 * (u % 2)
                vT_u = vT[pb : pb + 64, u // 2, :]
                for jg in range(NJT // VB):
                    ps = psAv.tile([P, VB, HD], f32, tag="psAv")
                    for k in range(VB):
                        jt = jg * VB + k
                        nc.tensor.transpose(
                            ps[:, k, :],
                            vT_u[:, jt * P : (jt + 1) * P],
                            ident[pb : pb + 64, pb : pb + 64],
                        )
                    cp(v1[:, u, jg * VB : jg * VB + VB, 0:HD], ps)

        # ======== scope B/C: attention + proj ========
        with tc.tile_pool(name="otpool", bufs=1) as otpool:
            OT = otpool.tile([P, 2, nseq], f32r)

            with (
                tc.tile_pool(name="epool", bufs=4) as epool,
                tc.tile_pool(name="obuf", bufs=2) as obuf,
                tc.tile_pool(name="small", bufs=1) as small,
                tc.tile_pool(name="psS", bufs=2, space="PSUM") as psS,
                tc.tile_pool(name="psO", bufs=4, space="PSUM") as psO,
            ):
                # ---- Phase 4: attention per unit ----
                # S(jt) runs one step ahead of O(jt-1) so the PE never
                # in-order-blocks on the exp of the current jt.
                for u in range(4):
                    pb = 64 * (u % 2)
                    qT_u = qkT[pb : pb + 64, u // 2, :]
                    kT_u = qkT[pb : pb + 64, 2 + u // 2, :]
                    psO_tiles = [
                        psO.tile([P, QCH], f32, tag="psO", name=f"psO_{u}_{q}")
                        for q in range(NOB)
                    ]

                    def emit_O(pjt, ets, u=u, psO_tiles=psO_tiles):
                        for sw in range(NSW):
                            for q2 in range(SW // QCH):
                                q = sw * (SW // QCH) + q2
                                nc.tensor.matmul(
                                    psO_tiles[q][0 : HD + 1, :],
                                    lhsT=v1[:, u, pjt, :],
                                    rhs=ets[sw][:, q2 * QCH : (q2 + 1) * QCH],
                                    start=(pjt == 0),
                                    stop=(pjt == NJT - 1),
                                )

                    prev = None
                    for jt in range(NJT):
                        ets = []
                        for sw in range(NSW):
                            ps = psS.tile([P, SW], f32, tag="psS")
                            for q2 in range(SW // QCH):
                                nc.tensor.matmul(
                                    ps[:, q2 * QCH : (q2 + 1) * QCH],
                                    lhsT=kT_u[:, jt * P : (jt + 1) * P],
                                    rhs=qT_u[
                                        :,
                                        sw * SW + q2 * QCH : sw * SW + (q2 + 1) * QCH,
                                    ],
                                    start=True,
                                    stop=True,
                                )
                            et = epool.tile([P, SW], f32r, tag="epool")
                            nc.scalar.activation(et, ps, EXP, scale=SCALE)
                            ets.append(et)
                        if prev is not None:
                            emit_O(jt - 1, prev)
                        prev = ets
                    emit_O(NJT - 1, prev)

                    # drain psO fast: copy O' and rowsum to SBUF, then
                    # normalize off the PSUM-release path.
                    o_sb = obuf.tile([64, nseq], f32, tag="obuf", name=f"o_sb_{u}")
                    rs_sb = small.tile([1, nseq], f32, tag="rs")
                    for q in range(NOB):
                        cp(o_sb[:, q * QCH : (q + 1) * QCH], psO_tiles[q][0:64, :])
                        cp(
                            rs_sb[:, q * QCH : (q + 1) * QCH],
                            psO_tiles[q][HD : HD + 1, :],
                        )
                    recip = small.tile([1, nseq], f32, tag="recip")
                    for q in range(NOB):
                        nc.vector.reciprocal(
                            recip[:, q * QCH : (q + 1) * QCH],
                            rs_sb[:, q * QCH : (q + 1) * QCH],
                        )
                    bcast = small.tile([64, nseq], f32, tag="bcast")
                    nc.gpsimd.partition_broadcast(bcast, recip)
                    for q in range(NOB):
                        nc.vector.tensor_mul(
                            OT[pb : pb + 64, u // 2, q * QCH : (q + 1) * QCH],
                            o_sb[:, q * QCH : (q + 1) * QCH],
                            bcast[:, q * QCH : (q + 1) * QCH],
                        )

            with (
                tc.tile_pool(name="opool", bufs=3) as opool,
                tc.tile_pool(name="psP", bufs=2, space="PSUM") as psP,
            ):
                # ---- Phase 5: proj partial out[i, e] ----
                for it in range(NIT):
                    for ech in range(C // ECH):
                        ps = psP.tile([P, ECH], f32, tag="psP")
                        for co in range(2):
                            nc.tensor.matmul(
                                ps,
                                lhsT=OT[:, co, it * P : (it + 1) * P],
                                rhs=wp_sb[:, co, ech * ECH : (ech + 1) * ECH],
                                start=(co == 0),
                                stop=(co == 1),
                            )
                        ot = opool.tile([P, ECH], f32, tag="opool")
                        cp(ot, ps)
                        nc.sync.dma_start(
                            out_d[it * P : (it + 1) * P, ech * ECH : (ech + 1) * ECH],
                            ot,
                        )

    nc.compile()
    return nc


def get_nc(nseq=NSEQ):
    if nseq not in _cache:
        _cache[nseq] = _build(nseq)
    return _cache[nseq]


def make_in_maps(x, w_qkv, w_proj, nseq=NSEQ):
    x = np.ascontiguousarray(x, dtype=np.float32)
    w_qkv = np.ascontiguousarray(w_qkv, dtype=np.float32)
    w_proj = np.ascontiguousarray(w_proj, dtype=np.float32)
    in_maps = []
    for core in range(8):
        b, hg = core // 4, core % 4
        hs = 4 * hg
        wsel = np.empty((6, P, C), np.float32)
        for mt in range(6):
            t, half = mt // 2, mt % 2
            r0 = t * C + (hs + 2 * half) * HD
            wsel[mt] = w_qkv[r0 : r0 + P, :]
        wqkvT = np.ascontiguousarray(wsel.transpose(2, 0, 1).reshape(C, 6 * P))
        wp = np.empty((P, 2, C), np.float32)
        for co in range(2):
            c0 = (hs + 2 * co) * HD
            wp[:, co, :] = w_proj[:, c0 : c0 + P].T
        in_maps.append(
            {"x": np.ascontiguousarray(x[b, :nseq]), "wqkvT": wqkvT, "wprojT": wp}
        )
    return in_maps


def kernel(x, w_qkv, w_proj, b_proj):
    from concourse.bass_utils import run_bass_kernel_spmd

    nc = get_nc()
    in_maps = make_in_maps(x, w_qkv, w_proj)
    res = run_bass_kernel_spmd(nc, in_maps, core_ids=list(range(8)))
    parts = [r["out"] for r in res.results]
    out = np.stack(
        [
            parts[0] + parts[1] + parts[2] + parts[3],
            parts[4] + parts[5] + parts[6] + parts[7],
        ],
        axis=0,
    )
    return (out + np.asarray(b_proj, np.float32)).astype(np.float32)


# revision 11
# speedup vs baseline: 1.2126x; 1.2126x over previous
# Multi-head attention (B=2, N=2048, C=1024, H=16) on 8 trn2 NeuronCores.
#
# Sharding: core = (batch b = core//4, head-group hg = core%4, 4 heads each).
# Each core computes qkv/attention/proj for its 4 heads of its batch and
# returns a partial projection output [N, C]; the host sums the 4 partials
# per batch and adds b_proj.
#
# Per-core device pipeline (all matmuls in float32r, full-rate at N>=256):
#   1. x [N,C] -> PE-transpose -> xT chunks [C, 512]          (f32r transposes)
#   2. qkvT[768, N] = Wsel @ x^T  (lhsT=wqkvT, rhs=xT chunk)  -> q^T,k^T,v^T
#      q^T/k^T stored per-unit zero-padded to K=128 partitions (keeps the
#      PE activity monitor warm during attention); v^T transposed back to
#      natural v' = [v | 1 | 0pad] right after each chunk.
#   3. per (head) unit: S^T[j,i] = k @ q^T (K=128 zero-padded);
#      E=exp(S*scale) on ACT; O'^T[128, N] += v'^T @ E^T (row 64 = rowsum,
#      rows 65.. = zero); S(jt) issued ahead of O(jt-1).
#      Drain: copy O'/rowsum to SBUF (frees PSUM fast), reciprocal +
#      gpsimd partition-broadcast + multiply off the critical path.
#   4. proj partial: out[i,e] = sum_ch O^T[ch,i] * wprojT[ch,e]
import sys

import numpy as np

if "/opt/trn_rl_repo" not in sys.path:
    sys.path.insert(0, "/opt/trn_rl_repo")

B, NSEQ, C = 2, 2048, 1024
H, HD = 16, 64
P = 128
SCALE = HD**-0.5

_cache = {}


def _build(nseq):
    from contextlib import ExitStack

    import concourse.tile as tile
    from concourse import bacc, mybir
    from concourse.masks import make_identity

    f32 = mybir.dt.float32
    f32r = mybir.dt.float32r
    EXP = mybir.ActivationFunctionType.Exp

    NJT = nseq // P          # j tiles (keys)
    NIT = nseq // P          # i tiles
    QCH = min(512, nseq)     # matmul moving-dim chunk
    NCH = nseq // QCH        # number of i chunks
    ITC = QCH // P           # i-tiles (and j-tiles) per chunk
    SW = min(1024, nseq)     # S^T psum tile width (2 banks)
    NSW = nseq // SW
    NOB = nseq // QCH        # number of O' psum tiles
    ECH = 512                # proj output chunk

    nc = bacc.Bacc("TRN2", target_bir_lowering=False, debug=False, num_devices=8)
    x_d = nc.dram_tensor("x", [nseq, C], f32r, kind="ExternalInput")
    wq_d = nc.dram_tensor("wqkvT", [C, 6 * P], f32r, kind="ExternalInput")
    wp_d = nc.dram_tensor("wprojT", [P, 2, C], f32r, kind="ExternalInput")
    out_d = nc.dram_tensor("out", [nseq, C], f32, kind="ExternalOutput")

    cp_state = [0]

    def cp(out, in_):
        # alternate PSUM->SBUF copies between DVE and ACT
        cp_state[0] ^= 1
        if cp_state[0]:
            nc.vector.tensor_copy(out, in_)
        else:
            nc.scalar.copy(out, in_)

    with tile.TileContext(nc) as tc, ExitStack() as ctx:
        persist = ctx.enter_context(tc.tile_pool(name="persist", bufs=1))
        qkpool = ctx.enter_context(tc.tile_pool(name="qkpool", bufs=1))
        v1pool = ctx.enter_context(tc.tile_pool(name="v1pool", bufs=1))

        wp_sb = persist.tile([P, 2, C], f32r)
        nc.gpsimd.dma_start(wp_sb, wp_d.ap())
        ones_f32 = persist.tile([P, 1], f32)
        nc.vector.memset(ones_f32, 1.0)
        zeros_f32 = persist.tile([P, 1], f32)
        nc.vector.memset(zeros_f32, 0.0)

        # q^T/k^T per unit, zero-padded to full 128 partitions.
        # slot u = q of unit u; slot 4+u = k of unit u.
        qk_sb = qkpool.tile([P, 8, nseq], f32r)
        # v' natural [j_part, u, jt, 128]: cols 0:64 v, col 64 ones, rest 0.
        v1 = v1pool.tile([P, 4, NJT, P], f32r)

        # zero-fill pad regions (engine copies round to f32r)
        for u in range(4):
            zpb = 64 if u % 2 == 0 else 0
            for slot in (u, 4 + u):
                nc.scalar.copy(
                    qk_sb[zpb : zpb + 64, slot, :],
                    zeros_f32[0:64, None, :].to_broadcast([64, 1, nseq]),
                )
        nc.vector.tensor_copy(
            v1[:, :, :, HD + 1 :],
            zeros_f32[:, None, None, :].to_broadcast([P, 4, NJT, P - HD - 1]),
        )
        nc.vector.tensor_copy(
            v1[:, :, :, HD : HD + 1],
            ones_f32[:, None, None, :].to_broadcast([P, 4, NJT, 1]),
        )

        # ======== scope A: transpose x, qkv matmuls, v' build ========
        with (
            tc.tile_pool(name="scopeA", bufs=1) as scopeA,
            tc.tile_pool(name="xin", bufs=2) as xin,
            tc.tile_pool(name="xtc", bufs=2) as xtc,
            tc.tile_pool(name="vtc", bufs=2) as vtc,
            tc.tile_pool(name="psA", bufs=3, space="PSUM") as psA,
            tc.tile_pool(name="psAv", bufs=2, space="PSUM") as psAv,
            tc.tile_pool(name="psQ", bufs=2, space="PSUM") as psQ,
        ):
            ident_f32 = scopeA.tile([P, P], f32, name="ident_f32")
            make_identity(nc, ident_f32)
            identR = scopeA.tile([P, P], f32r)
            nc.vector.tensor_copy(identR, ident_f32)
            wq_sb = scopeA.tile([P, 8, 6 * P], f32r)
            nc.gpsimd.dma_start(wq_sb, wq_d.ap().rearrange("(co p) d -> p co d", p=P))

            for nch in range(NCH):
                # ---- transpose x chunk into xT[c_part, c_outer, i_chunk] ----
                xT = xtc.tile([P, 8, QCH], f32r, tag="xtc")
                for itl in range(ITC):
                    it = nch * ITC + itl
                    xt = xin.tile([P, C], f32r, tag="xt")
                    nc.sync.dma_start(xt, x_d[it * P : (it + 1) * P, :])
                    for cg in range(2):
                        ps = psA.tile([P, 4, P], f32r, tag="psA")
                        for k in range(4):
                            cch = cg * 4 + k
                            nc.tensor.transpose(
                                ps[:, k, :], xt[:, cch * P : (cch + 1) * P], identR
                            )
                        cp(
                            xT[:, cg * 4 : cg * 4 + 4, itl * P : (itl + 1) * P],
                            ps,
                        )

                # ---- qkv matmuls for this chunk ----
                vT = vtc.tile([P, 2, QCH], f32, tag="vtc")
                for mt in range(6):
                    ps = psQ.tile([P, QCH], f32, tag="psQ")
                    for co in range(8):
                        nc.tensor.matmul(
                            ps,
                            lhsT=wq_sb[:, co, mt * P : (mt + 1) * P],
                            rhs=xT[:, co, :],
                            start=(co == 0),
                            stop=(co == 7),
                        )
                    if mt < 4:
                        # rows 0:64 = unit 2*half, rows 64:128 = unit 2*half+1
                        half = mt % 2
                        base = 0 if mt < 2 else 4
                        sl = slice(nch * QCH, (nch + 1) * QCH)
                        cp(qk_sb[0:64, base + 2 * half, sl], ps[0:64, :])
                        cp(qk_sb[64:128, base + 2 * half + 1, sl], ps[64:128, :])
                    else:
                        cp(vT[:, mt - 4, :], ps)

                # ---- v' natural for this chunk's j-tiles ----
                for u in range(4):
                    pb = 64 * (u % 2)
                    vT_u = vT[pb : pb + 64, u // 2, :]
                    ps = psAv.tile([P, ITC, HD], f32, tag="psAv")
                    for k in range(ITC):
                        nc.tensor.transpose(
                            ps[:, k, :],
                            vT_u[:, k * P : (k + 1) * P],
                            ident_f32[pb : pb + 64, pb : pb + 64],
                        )
                    cp(v1[:, u, nch * ITC : (nch + 1) * ITC, 0:HD], ps)

        # ======== scope B/C: attention + proj ========
        with tc.tile_pool(name="otpool", bufs=1) as otpool:
            OT = otpool.tile([P, 2, nseq], f32r)

            with (
                tc.tile_pool(name="epool", bufs=4) as epool,
                tc.tile_pool(name="obuf", bufs=2) as obuf,
                tc.tile_pool(name="small", bufs=1) as small,
                tc.tile_pool(name="psS", bufs=2, space="PSUM") as psS,
                tc.tile_pool(name="psO", bufs=4, space="PSUM") as psO,
            ):
                # ---- attention per unit ----
                # S(jt) runs one step ahead of O(jt-1) so the PE never
                # in-order-blocks on the exp of the current jt.
                for u in range(4):
                    pb = 64 * (u % 2)
                    qT_u = qk_sb[:, u, :]
                    kT_u = qk_sb[:, 4 + u, :]
                    psO_tiles = [
                        psO.tile([P, QCH], f32, tag="psO", name=f"psO_{u}_{q}")
                        for q in range(NOB)
                    ]

                    def emit_O(pjt, ets, u=u, psO_tiles=psO_tiles):
                        for sw in range(NSW):
                            for q2 in range(SW // QCH):
                                q = sw * (SW // QCH) + q2
                                nc.tensor.matmul(
                                    psO_tiles[q],
                                    lhsT=v1[:, u, pjt, :],
                                    rhs=ets[sw][:, q2 * QCH : (q2 + 1) * QCH],
                                    start=(pjt == 0),
                                    stop=(pjt == NJT - 1),
                                )

                    prev = None
                    for jt in range(NJT):
                        ets = []
                        for sw in range(NSW):
                            ps = psS.tile([P, SW], f32, tag="psS")
                            for q2 in range(SW // QCH):
                                nc.tensor.matmul(
                                    ps[:, q2 * QCH : (q2 + 1) * QCH],
                                    lhsT=kT_u[:, jt * P : (jt + 1) * P],
                                    rhs=qT_u[
                                        :,
                                        sw * SW + q2 * QCH : sw * SW + (q2 + 1) * QCH,
                                    ],
                                    start=True,
                                    stop=True,
                                )
                            et = epool.tile([P, SW], f32r, tag="epool")
                            nc.scalar.activation(et, ps, EXP, scale=SCALE)
                            ets.append(et)
                        if prev is not None:
                            emit_O(jt - 1, prev)
                        prev = ets
                    emit_O(NJT - 1, prev)

                    # drain psO fast; normalize off the PSUM-release path
                    o_sb = obuf.tile([64, nseq], f32, tag="obuf", name=f"o_sb_{u}")
                    rs_sb = small.tile([1, nseq], f32, tag="rs")
                    for q in range(NOB):
                        cp(o_sb[:, q * QCH : (q + 1) * QCH], psO_tiles[q][0:64, :])
                        cp(
                            rs_sb[:, q * QCH : (q + 1) * QCH],
                            psO_tiles[q][HD : HD + 1, :],
                        )
                    recip = small.tile([1, nseq], f32, tag="recip")
                    for q in range(NOB):
                        nc.vector.reciprocal(
                            recip[:, q * QCH : (q + 1) * QCH],
                            rs_sb[:, q * QCH : (q + 1) * QCH],
                        )
                    bcast = small.tile([64, nseq], f32, tag="bcast")
                    nc.gpsimd.partition_broadcast(bcast, recip)
                    for q in range(NOB):
                        nc.vector.tensor_mul(
                            OT[pb : pb + 64, u // 2, q * QCH : (q + 1) * QCH],
                            o_sb[:, q * QCH : (q + 1) * QCH],
                            bcast[:, q * QCH : (q + 1) * QCH],
                        )

            with (
                tc.tile_pool(name="opool", bufs=3) as opool,
                tc.tile_pool(name="psP", bufs=2, space="PSUM") as psP,
            ):
                # ---- proj partial out[i, e] ----
                for it in range(NIT):
                    for ech in range(C // ECH):
                        ps = psP.tile([P, ECH], f32, tag="psP")
                        for co in range(2):
                            nc.tensor.matmul(
                                ps,
                                lhsT=OT[:, co, it * P : (it + 1) * P],
                                rhs=wp_sb[:, co, ech * ECH : (ech + 1) * ECH],
                                start=(co == 0),
                                stop=(co == 1),
                            )
                        ot = opool.tile([P, ECH], f32, tag="opool")
                        cp(ot, ps)
                        nc.sync.dma_start(
                            out_d[it * P : (it + 1) * P, ech * ECH : (ech + 1) * ECH],
                            ot,
                        )

    nc.compile()
    return nc


def get_nc(nseq=NSEQ):
    if nseq not in _cache:
        _cache[nseq] = _build(nseq)
    return _cache[nseq]


def make_in_maps(x, w_qkv, w_proj, nseq=NSEQ):
    x = np.ascontiguousarray(x, dtype=np.float32)
    w_qkv = np.ascontiguousarray(w_qkv, dtype=np.float32)
    w_proj = np.ascontiguousarray(w_proj, dtype=np.float32)
    in_maps = []
    for core in range(8):
        b, hg = core // 4, core % 4
        hs = 4 * hg
        wsel = np.empty((6, P, C), np.float32)
        for mt in range(6):
            t, half = mt // 2, mt % 2
            r0 = t * C + (hs + 2 * half) * HD
            wsel[mt] = w_qkv[r0 : r0 + P, :]
        wqkvT = np.ascontiguousarray(wsel.transpose(2, 0, 1).reshape(C, 6 * P))
        wp = np.empty((P, 2, C), np.float32)
        for co in range(2):
            c0 = (hs + 2 * co) * HD
            wp[:, co, :] = w_proj[:, c0 : c0 + P].T
        in_maps.append(
            {"x": np.ascontiguousarray(x[b, :nseq]), "wqkvT": wqkvT, "wprojT": wp}
        )
    return in_maps


def kernel(x, w_qkv, w_proj, b_proj):
    from concourse.bass_utils import run_bass_kernel_spmd

    nc = get_nc()
    in_maps = make_in_maps(x, w_qkv, w_proj)
    res = run_bass_kernel_spmd(nc, in_maps, core_ids=list(range(8)))
    parts = [r["out"] for r in res.results]
    out = np.stack(
        [
            parts[0] + parts[1] + parts[2] + parts[3],
            parts[4] + parts[5] + parts[6] + parts[7],
        ],
        axis=0,
    )
    return (out + np.asarray(b_proj, np.float32)).astype(np.float32)


# revision 12
# speedup vs baseline: 1.5459x; 1.2749x over previous
# Multi-head attention (B=2, N=2048, C=1024, H=16) on 8 trn2 NeuronCores.
#
# Sharding: core = (batch b = core//4, head-group hg = core%4, 4 heads each).
# Each core computes qkv/attention/proj for its 4 heads of its batch and
# returns a partial projection output [N, C]; the host sums the 4 partials
# per batch and adds b_proj.
#
# Per-core device pipeline (all matmuls in float32r, full-rate at N>=256):
#   1. x [N,C] -> PE-transpose -> xT chunks [C, 512]          (f32r transposes)
#   2. qkvT[768, N] = Wsel @ x^T  (lhsT=wqkvT, rhs=xT chunk)  -> q^T,k^T,v^T
#      q^T/k^T stored per-unit zero-padded to K=128 partitions (keeps the
#      PE activity monitor warm during attention); v^T transposed back to
#      natural v' = [v | 1 | 0pad] right after each chunk.
#   3. per (head) unit: S^T[j,i] = k @ q^T (K=128 zero-padded);
#      E=exp(S*scale) on ACT; O'^T[128, N] += v'^T @ E^T (row 64 = rowsum,
#      rows 65.. = zero); S(jt) issued ahead of O(jt-1).
#      Drain: copy O'/rowsum to SBUF (frees PSUM fast), reciprocal +
#      gpsimd partition-broadcast + multiply off the critical path.
#   4. proj partial: out[i,e] = sum_ch O^T[ch,i] * wprojT[ch,e]
import sys

import numpy as np

if "/opt/trn_rl_repo" not in sys.path:
    sys.path.insert(0, "/opt/trn_rl_repo")

B, NSEQ, C = 2, 2048, 1024
H, HD = 16, 64
P = 128
SCALE = HD**-0.5

_cache = {}


def _build(nseq):
    from contextlib import ExitStack

    import concourse.tile as tile
    from concourse import bacc, mybir
    from concourse.masks import make_identity

    f32 = mybir.dt.float32
    f32r = mybir.dt.float32r
    EXP = mybir.ActivationFunctionType.Exp

    NJT = nseq // P          # j tiles (keys)
    NIT = nseq // P          # i tiles
    QCH = min(512, nseq)     # matmul moving-dim chunk
    NCH = nseq // QCH        # number of i chunks
    ITC = QCH // P           # i-tiles (and j-tiles) per chunk
    SW = min(1024, nseq)     # S^T psum tile width (2 banks)
    NSW = nseq // SW
    NOB = nseq // QCH        # number of O' psum tiles
    ECH = 512                # proj output chunk

    nc = bacc.Bacc("TRN2", target_bir_lowering=False, debug=False, num_devices=8)
    x_d = nc.dram_tensor("x", [nseq, C], f32r, kind="ExternalInput")
    wq_d = nc.dram_tensor("wqkvT", [C, 6 * P], f32r, kind="ExternalInput")
    wp_d = nc.dram_tensor("wprojT", [P, 2, C], f32r, kind="ExternalInput")
    out_d = nc.dram_tensor("out", [nseq, C], f32, kind="ExternalOutput")

    cp_state = [0]

    def cp(out, in_):
        # alternate PSUM->SBUF copies between DVE and ACT
        cp_state[0] ^= 1
        if cp_state[0]:
            nc.vector.tensor_copy(out, in_)
        else:
            nc.scalar.copy(out, in_)

    with tile.TileContext(nc) as tc, ExitStack() as ctx:
        persist = ctx.enter_context(tc.tile_pool(name="persist", bufs=1))
        qkpool = ctx.enter_context(tc.tile_pool(name="qkpool", bufs=1))
        v1pool = ctx.enter_context(tc.tile_pool(name="v1pool", bufs=1))

        wp_sb = persist.tile([P, 2, C], f32r)
        nc.gpsimd.dma_start(wp_sb, wp_d.ap())
        ones_f32 = persist.tile([P, 1], f32)
        nc.vector.memset(ones_f32, 1.0)
        zeros_f32 = persist.tile([P, 1], f32)
        nc.vector.memset(zeros_f32, 0.0)

        # q^T/k^T per unit, zero-padded to full 128 partitions.
        # slot u = q of unit u; slot 4+u = k of unit u.
        qk_sb = qkpool.tile([P, 8, nseq], f32r)
        # v' natural [j_part, u, jt, 128]: cols 0:64 v, col 64 ones, rest 0.
        v1 = v1pool.tile([P, 4, NJT, P], f32r)

        # zero-fill pad regions (engine copies round to f32r)
        for u in range(4):
            zpb = 64 if u % 2 == 0 else 0
            for slot in (u, 4 + u):
                nc.vector.tensor_copy(
                    qk_sb[zpb : zpb + 64, slot, :],
                    zeros_f32[0:64, None, :].to_broadcast([64, 1, nseq]),
                )
        # prime the ACT exp table early so unit 0 doesn't stall on it
        prime = persist.tile([P, 1], f32)
        nc.scalar.activation(prime, ones_f32, EXP, scale=0.0)
        nc.vector.tensor_copy(
            v1[:, :, :, HD + 1 :],
            zeros_f32[:, None, None, :].to_broadcast([P, 4, NJT, P - HD - 1]),
        )
        nc.vector.tensor_copy(
            v1[:, :, :, HD : HD + 1],
            ones_f32[:, None, None, :].to_broadcast([P, 4, NJT, 1]),
        )

        # ======== scope A: transpose x, qkv matmuls, v' build ========
        with (
            tc.tile_pool(name="scopeA", bufs=1) as scopeA,
            tc.tile_pool(name="xin", bufs=2) as xin,
            tc.tile_pool(name="xtc", bufs=2) as xtc,
            tc.tile_pool(name="vtc", bufs=2) as vtc,
            tc.tile_pool(name="psA", bufs=3, space="PSUM") as psA,
            tc.tile_pool(name="psAv", bufs=2, space="PSUM") as psAv,
            tc.tile_pool(name="psQ", bufs=2, space="PSUM") as psQ,
        ):
            ident_f32 = scopeA.tile([P, P], f32, name="ident_f32")
            make_identity(nc, ident_f32)
            identR = scopeA.tile([P, P], f32r)
            nc.vector.tensor_copy(identR, ident_f32)
            wq_sb = scopeA.tile([P, 8, 6 * P], f32r)
            nc.gpsimd.dma_start(wq_sb, wq_d.ap().rearrange("(co p) d -> p co d", p=P))

            # Transposes are sprinkled between qkv matmul bursts: PE
            # transpose-mode does not register as activity for the PE clock
            # monitor, so long transpose-only stretches re-throttle the PE
            # clock to 1.2 GHz. Interleaving keeps matmul duty high.
            xT_tiles = {}
            xt_tiles = {}
            vT_tiles = {}

            def x_group(nch, itl, cg):
                it = nch * ITC + itl
                if itl == 0 and cg == 0:
                    xT_tiles[nch] = xtc.tile(
                        [P, 8, QCH], f32r, tag="xtc", name=f"xT_{nch}"
                    )
                xT = xT_tiles[nch]
                if cg == 0:
                    xt = xin.tile([P, C], f32r, tag="xt", name=f"xt_{it}")
                    xt_tiles[it] = xt
                    nc.sync.dma_start(xt, x_d[it * P : (it + 1) * P, :])
                xt = xt_tiles[it]
                ps = psA.tile([P, 4, P], f32r, tag="psA")
                for k in range(4):
                    cch = cg * 4 + k
                    nc.tensor.transpose(
                        ps[:, k, :], xt[:, cch * P : (cch + 1) * P], identR
                    )
                cp(xT[:, cg * 4 : cg * 4 + 4, itl * P : (itl + 1) * P], ps)

            def v_group(nch, u):
                vT = vT_tiles[nch]
                pb = 64 * (u % 2)
                vT_u = vT[pb : pb + 64, u // 2, :]
                ps = psAv.tile([P, ITC, HD], f32, tag="psAv")
                for k in range(ITC):
                    nc.tensor.transpose(
                        ps[:, k, :],
                        vT_u[:, k * P : (k + 1) * P],
                        ident_f32[pb : pb + 64, pb : pb + 64],
                    )
                cp(v1[:, u, nch * ITC : (nch + 1) * ITC, 0:HD], ps)

            for itl in range(ITC):
                for cg in range(2):
                    x_group(0, itl, cg)

            for nch in range(NCH):
                sprinkle = []
                if nch + 1 < NCH:
                    sprinkle += [
                        (x_group, (nch + 1, itl, cg))
                        for itl in range(ITC)
                        for cg in range(2)
                    ]
                if nch >= 1:
                    sprinkle += [(v_group, (nch - 1, u)) for u in range(4)]
                vT_tiles[nch] = vtc.tile(
                    [P, 2, QCH], f32, tag="vtc", name=f"vT_{nch}"
                )
                vT = vT_tiles[nch]
                xT = xT_tiles[nch]
                per_gap = -(-len(sprinkle) // 6) if sprinkle else 0
                si = 0
                for mt in range(6):
                    ps = psQ.tile([P, QCH], f32, tag="psQ")
                    for co in range(8):
                        nc.tensor.matmul(
                            ps,
                            lhsT=wq_sb[:, co, mt * P : (mt + 1) * P],
                            rhs=xT[:, co, :],
                            start=(co == 0),
                            stop=(co == 7),
                        )
                    if mt < 4:
                        # rows 0:64 = unit 2*half, rows 64:128 = unit 2*half+1
                        half = mt % 2
                        base = 0 if mt < 2 else 4
                        sl = slice(nch * QCH, (nch + 1) * QCH)
                        cp(qk_sb[0:64, base + 2 * half, sl], ps[0:64, :])
                        cp(qk_sb[64:128, base + 2 * half + 1, sl], ps[64:128, :])
                    else:
                        cp(vT[:, mt - 4, :], ps)
                    for _ in range(per_gap):
                        if si < len(sprinkle):
                            f, a = sprinkle[si]
                            f(*a)
                            si += 1
                del xT_tiles[nch]
            for u in range(4):
                v_group(NCH - 1, u)

        # ======== scope B/C: attention + proj ========
        with tc.tile_pool(name="otpool", bufs=1) as otpool:
            OT = otpool.tile([P, 2, nseq], f32r)

            with (
                tc.tile_pool(name="epool", bufs=4) as epool,
                tc.tile_pool(name="obuf", bufs=2) as obuf,
                tc.tile_pool(name="small", bufs=1) as small,
                tc.tile_pool(name="psS", bufs=2, space="PSUM") as psS,
                tc.tile_pool(name="psO", bufs=4, space="PSUM") as psO,
            ):
                # ---- attention per unit ----
                # S(jt) runs one step ahead of O(jt-1) so the PE never
                # in-order-blocks on the exp of the current jt.
                for u in range(4):
                    pb = 64 * (u % 2)
                    qT_u = qk_sb[:, u, :]
                    kT_u = qk_sb[:, 4 + u, :]
                    psO_tiles = [
                        psO.tile([P, QCH], f32, tag="psO", name=f"psO_{u}_{q}")
                        for q in range(NOB)
                    ]

                    def emit_O(pjt, ets, u=u, psO_tiles=psO_tiles):
                        for sw in range(NSW):
                            for q2 in range(SW // QCH):
                                q = sw * (SW // QCH) + q2
                                nc.tensor.matmul(
                                    psO_tiles[q],
                                    lhsT=v1[:, u, pjt, :],
                                    rhs=ets[sw][:, q2 * QCH : (q2 + 1) * QCH],
                                    start=(pjt == 0),
                                    stop=(pjt == NJT - 1),
                                )

                    prev = None
                    for jt in range(NJT):
                        ets = []
                        for sw in range(NSW):
                            ps = psS.tile([P, SW], f32, tag="psS")
                            for q2 in range(SW // QCH):
                                nc.tensor.matmul(
                                    ps[:, q2 * QCH : (q2 + 1) * QCH],
                                    lhsT=kT_u[:, jt * P : (jt + 1) * P],
                                    rhs=qT_u[
                                        :,
                                        sw * SW + q2 * QCH : sw * SW + (q2 + 1) * QCH,
                                    ],
                                    start=True,
                                    stop=True,
                                )
                            et = epool.tile([P, SW], f32r, tag="epool")
                            nc.scalar.activation(et, ps, EXP, scale=SCALE)
                            ets.append(et)
                        if prev is not None:
                            emit_O(jt - 1, prev)
                        prev = ets
                    emit_O(NJT - 1, prev)

                    # drain psO fast; normalize off the PSUM-release path
                    o_sb = obuf.tile([64, nseq], f32, tag="obuf", name=f"o_sb_{u}")
                    rs_sb = small.tile([1, nseq], f32, tag="rs")
                    for q in range(NOB):
                        nc.vector.tensor_copy(
                            o_sb[:, q * QCH : (q + 1) * QCH], psO_tiles[q][0:64, :]
                        )
                        nc.vector.tensor_copy(
                            rs_sb[:, q * QCH : (q + 1) * QCH],
                            psO_tiles[q][HD : HD + 1, :],
                        )
                    recip = small.tile([1, nseq], f32, tag="recip")
                    for q in range(NOB):
                        nc.vector.reciprocal(
                            recip[:, q * QCH : (q + 1) * QCH],
                            rs_sb[:, q * QCH : (q + 1) * QCH],
                        )
                    bcast = small.tile([64, nseq], f32, tag="bcast")
                    nc.gpsimd.partition_broadcast(bcast, recip)
                    for q in range(NOB):
                        nc.vector.tensor_mul(
                            OT[pb : pb + 64, u // 2, q * QCH : (q + 1) * QCH],
                            o_sb[:, q * QCH : (q + 1) * QCH],
                            bcast[:, q * QCH : (q + 1) * QCH],
                        )

            with (
                tc.tile_pool(name="opool", bufs=3) as opool,
                tc.tile_pool(name="psP", bufs=2, space="PSUM") as psP,
            ):
                # ---- proj partial out[i, e] ----
                for it in range(NIT):
                    for ech in range(C // ECH):
                        ps = psP.tile([P, ECH], f32, tag="psP")
                        for co in range(2):
                            nc.tensor.matmul(
                                ps,
                                lhsT=OT[:, co, it * P : (it + 1) * P],
                                rhs=wp_sb[:, co, ech * ECH : (ech + 1) * ECH],
                                start=(co == 0),
                                stop=(co == 1),
                            )
                        ot = opool.tile([P, ECH], f32, tag="opool")
                        cp(ot, ps)
                        nc.sync.dma_start(
                            out_d[it * P : (it + 1) * P, ech * ECH : (ech + 1) * ECH],
                            ot,
                        )

    nc.compile()
    return nc


def get_nc(nseq=NSEQ):
    if nseq not in _cache:
        _cache[nseq] = _build(nseq)
    return _cache[nseq]


def make_in_maps(x, w_qkv, w_proj, nseq=NSEQ):
    x = np.ascontiguousarray(x, dtype=np.float32)
    w_qkv = np.ascontiguousarray(w_qkv, dtype=np.float32)
    w_proj = np.ascontiguousarray(w_proj, dtype=np.float32)
    in_maps = []
    for core in range(8):
        b, hg = core // 4, core % 4
        hs = 4 * hg
        wsel = np.empty((6, P, C), np.float32)
        for mt in range(6):
            t, half = mt // 2, mt % 2
            r0 = t * C + (hs + 2 * half) * HD
            wsel[mt] = w_qkv[r0 : r0 + P, :]
        wqkvT = np.ascontiguousarray(wsel.transpose(2, 0, 1).reshape(C, 6 * P))
        wp = np.empty((P, 2, C), np.float32)
        for co in range(2):
            c0 = (hs + 2 * co) * HD
            wp[:, co, :] = w_proj[:, c0 : c0 + P].T
        in_maps.append(
            {"x": np.ascontiguousarray(x[b, :nseq]), "wqkvT": wqkvT, "wprojT": wp}
        )
    return in_maps


def kernel(x, w_qkv, w_proj, b_proj):
    from concourse.bass_utils import run_bass_kernel_spmd

    nc = get_nc()
    in_maps = make_in_maps(x, w_qkv, w_proj)
    res = run_bass_kernel_spmd(nc, in_maps, core_ids=list(range(8)))
    parts = [r["out"] for r in res.results]
    out = np.stack(
        [
            parts[0] + parts[1] + parts[2] + parts[3],
            parts[4] + parts[5] + parts[6] + parts[7],
        ],
        axis=0,
    )
    return (out + np.asarray(b_proj, np.float32)).astype(np.float32)


# revision 13
# speedup vs baseline: 1.6127x; 1.0432x over previous
# Multi-head attention (B=2, N=2048, C=1024, H=16) on 8 trn2 NeuronCores.
#
# Sharding: core = (batch b = core//4, head-group hg = core%4, 4 heads each).
# Each core computes qkv/attention/proj for its 4 heads of its batch and
# returns a partial projection output [N, C]; the host sums the 4 partials
# per batch and adds b_proj.
#
# Per-core device pipeline (all matmuls in float32r, full-rate at N>=256):
#   1. x [N,C] -> PE-transpose -> xT chunks [C, 512]          (f32r transposes)
#   2. qkvT[768, N] = Wsel @ x^T  (lhsT=wqkvT, rhs=xT chunk)  -> q^T,k^T,v^T
#      q^T/k^T stored per-unit zero-padded to K=128 partitions (keeps the
#      PE activity monitor warm during attention); v^T transposed back to
#      natural v' = [v | 1 | 0pad] right after each chunk.
#   3. per (head) unit: S^T[j,i] = k @ q^T (K=128 zero-padded);
#      E=exp(S*scale) on ACT; O'^T[128, N] += v'^T @ E^T (row 64 = rowsum,
#      rows 65.. = zero); S(jt) issued ahead of O(jt-1).
#      Drain: copy O'/rowsum to SBUF (frees PSUM fast), reciprocal +
#      gpsimd partition-broadcast + multiply off the critical path.
#   4. proj partial: out[i,e] = sum_ch O^T[ch,i] * wprojT[ch,e]
import sys

import numpy as np

if "/opt/trn_rl_repo" not in sys.path:
    sys.path.insert(0, "/opt/trn_rl_repo")

B, NSEQ, C = 2, 2048, 1024
H, HD = 16, 64
P = 128
SCALE = HD**-0.5

_cache = {}


def _build(nseq):
    from contextlib import ExitStack

    import concourse.tile as tile
    from concourse import bacc, mybir
    from concourse.masks import make_identity

    f32 = mybir.dt.float32
    f32r = mybir.dt.float32r
    EXP = mybir.ActivationFunctionType.Exp

    NJT = nseq // P          # j tiles (keys)
    NIT = nseq // P          # i tiles
    QCH = min(512, nseq)     # matmul moving-dim chunk
    NCH = nseq // QCH        # number of i chunks
    ITC = QCH // P           # i-tiles (and j-tiles) per chunk
    SW = min(1024, nseq)     # S^T psum tile width (2 banks)
    NSW = nseq // SW
    NOB = nseq // QCH        # number of O' psum tiles
    ECH = 512                # proj output chunk

    nc = bacc.Bacc("TRN2", target_bir_lowering=False, debug=False, num_devices=8)
    x_d = nc.dram_tensor("x", [nseq, C], f32r, kind="ExternalInput")
    wq_d = nc.dram_tensor("wqkvT", [C, 6 * P], f32r, kind="ExternalInput")
    wp_d = nc.dram_tensor("wprojT", [P, 2, C], f32r, kind="ExternalInput")
    out_d = nc.dram_tensor("out", [nseq, C], f32, kind="ExternalOutput")

    cp_state = [0]

    def cp(out, in_):
        # alternate PSUM->SBUF copies between DVE and ACT
        cp_state[0] ^= 1
        if cp_state[0]:
            nc.vector.tensor_copy(out, in_)
        else:
            nc.scalar.copy(out, in_)

    def cpA(out, in_):
        # ACT-only copy: phases 1-3 keep DVE free for the pad zero-fills
        nc.scalar.copy(out, in_)

    with tile.TileContext(nc) as tc, ExitStack() as ctx:
        persist = ctx.enter_context(tc.tile_pool(name="persist", bufs=1))
        qkpool = ctx.enter_context(tc.tile_pool(name="qkpool", bufs=1))
        v1pool = ctx.enter_context(tc.tile_pool(name="v1pool", bufs=1))

        wp_sb = persist.tile([P, 2, C], f32r)
        nc.gpsimd.dma_start(wp_sb, wp_d.ap())
        ones_f32 = persist.tile([P, 1], f32)
        nc.vector.memset(ones_f32, 1.0)
        zeros_f32 = persist.tile([P, 1], f32)
        nc.vector.memset(zeros_f32, 0.0)

        # q^T/k^T per unit, zero-padded to full 128 partitions.
        # slot u = q of unit u; slot 4+u = k of unit u.
        qk_sb = qkpool.tile([P, 8, nseq], f32r)
        # v' natural [j_part, u, jt, 128]: cols 0:64 v, col 64 ones, rest 0.
        v1 = v1pool.tile([P, 4, NJT, P], f32r)

        # prime the ACT exp table early so unit 0 doesn't stall on it
        prime = persist.tile([P, 1], f32)
        nc.scalar.activation(prime, ones_f32, EXP, scale=0.0)

        # ======== scope A: transpose x, qkv matmuls, v' build ========
        with (
            tc.tile_pool(name="scopeA", bufs=1) as scopeA,
            tc.tile_pool(name="xin", bufs=2) as xin,
            tc.tile_pool(name="xtc", bufs=2) as xtc,
            tc.tile_pool(name="vtc", bufs=2) as vtc,
            tc.tile_pool(name="psA", bufs=3, space="PSUM") as psA,
            tc.tile_pool(name="psAv", bufs=2, space="PSUM") as psAv,
            tc.tile_pool(name="psQ", bufs=2, space="PSUM") as psQ,
        ):
            ident_f32 = scopeA.tile([P, P], f32, name="ident_f32")
            make_identity(nc, ident_f32)
            identR = scopeA.tile([P, P], f32r)
            nc.vector.tensor_copy(identR, ident_f32)
            # pad zero-fills on DVE (after identR so transposes aren't blocked)
            for u in range(4):
                zpb = 64 if u % 2 == 0 else 0
                for slot in (u, 4 + u):
                    nc.vector.tensor_copy(
                        qk_sb[zpb : zpb + 64, slot, :],
                        zeros_f32[0:64, None, :].to_broadcast([64, 1, nseq]),
                    )
            nc.vector.tensor_copy(
                v1[:, :, :, HD + 1 :],
                zeros_f32[:, None, None, :].to_broadcast([P, 4, NJT, P - HD - 1]),
            )
            nc.vector.tensor_copy(
                v1[:, :, :, HD : HD + 1],
                ones_f32[:, None, None, :].to_broadcast([P, 4, NJT, 1]),
            )
            wq_sb = scopeA.tile([P, 8, 6 * P], f32r)
            nc.gpsimd.dma_start(wq_sb, wq_d.ap().rearrange("(co p) d -> p co d", p=P))

            # Transposes are sprinkled between qkv matmul bursts: PE
            # transpose-mode does not register as activity for the PE clock
            # monitor, so long transpose-only stretches re-throttle the PE
            # clock to 1.2 GHz. Interleaving keeps matmul duty high.
            xT_tiles = {}
            xt_tiles = {}
            vT_tiles = {}

            def x_group(nch, itl, cg):
                it = nch * ITC + itl
                if itl == 0 and cg == 0:
                    xT_tiles[nch] = xtc.tile(
                        [P, 8, QCH], f32r, tag="xtc", name=f"xT_{nch}"
                    )
                xT = xT_tiles[nch]
                if cg == 0:
                    xt = xin.tile([P, C], f32r, tag="xt", name=f"xt_{it}")
                    xt_tiles[it] = xt
                    nc.sync.dma_start(xt, x_d[it * P : (it + 1) * P, :])
                xt = xt_tiles[it]
                ps = psA.tile([P, 4, P], f32r, tag="psA")
                for k in range(4):
                    cch = cg * 4 + k
                    nc.tensor.transpose(
                        ps[:, k, :], xt[:, cch * P : (cch + 1) * P], identR
                    )
                cpA(xT[:, cg * 4 : cg * 4 + 4, itl * P : (itl + 1) * P], ps)

            def v_group(nch, u):
                vT = vT_tiles[nch]
                pb = 64 * (u % 2)
                vT_u = vT[pb : pb + 64, u // 2, :]
                ps = psAv.tile([P, ITC, HD], f32, tag="psAv")
                for k in range(ITC):
                    nc.tensor.transpose(
                        ps[:, k, :],
                        vT_u[:, k * P : (k + 1) * P],
                        ident_f32[pb : pb + 64, pb : pb + 64],
                    )
                cpA(v1[:, u, nch * ITC : (nch + 1) * ITC, 0:HD], ps)

            for itl in range(ITC):
                for cg in range(2):
                    x_group(0, itl, cg)

            for nch in range(NCH):
                sprinkle = []
                if nch + 1 < NCH:
                    sprinkle += [
                        (x_group, (nch + 1, itl, cg))
                        for itl in range(ITC)
                        for cg in range(2)
                    ]
                if nch >= 1:
                    sprinkle += [(v_group, (nch - 1, u)) for u in range(4)]
                vT_tiles[nch] = vtc.tile(
                    [P, 2, QCH], f32, tag="vtc", name=f"vT_{nch}"
                )
                vT = vT_tiles[nch]
                xT = xT_tiles[nch]
                per_gap = -(-len(sprinkle) // 6) if sprinkle else 0
                si = 0
                for mt in range(6):
                    ps = psQ.tile([P, QCH], f32, tag="psQ")
                    for co in range(8):
                        nc.tensor.matmul(
                            ps,
                            lhsT=wq_sb[:, co, mt * P : (mt + 1) * P],
                            rhs=xT[:, co, :],
                            start=(co == 0),
                            stop=(co == 7),
                        )
                    if mt < 4:
                        # rows 0:64 = unit 2*half, rows 64:128 = unit 2*half+1
                        half = mt % 2
                        base = 0 if mt < 2 else 4
                        sl = slice(nch * QCH, (nch + 1) * QCH)
                        cpA(qk_sb[0:64, base + 2 * half, sl], ps[0:64, :])
                        cpA(qk_sb[64:128, base + 2 * half + 1, sl], ps[64:128, :])
                    else:
                        cpA(vT[:, mt - 4, :], ps)
                    for _ in range(per_gap):
                        if si < len(sprinkle):
                            f, a = sprinkle[si]
                            f(*a)
                            si += 1
                del xT_tiles[nch]
            for u in range(4):
                v_group(NCH - 1, u)

        # ======== scope B/C: attention + proj ========
        with tc.tile_pool(name="otpool", bufs=1) as otpool:
            OT = otpool.tile([P, 2, nseq], f32r)

            with (
                tc.tile_pool(name="epool", bufs=4) as epool,
                tc.tile_pool(name="obuf", bufs=2) as obuf,
                tc.tile_pool(name="small", bufs=1) as small,
                tc.tile_pool(name="psS", bufs=2, space="PSUM") as psS,
                tc.tile_pool(name="psO", bufs=4, space="PSUM") as psO,
            ):
                # ---- attention per unit ----
                # S(jt) runs one step ahead of O(jt-1) so the PE never
                # in-order-blocks on the exp of the current jt.
                for u in range(4):
                    pb = 64 * (u % 2)
                    qT_u = qk_sb[:, u, :]
                    kT_u = qk_sb[:, 4 + u, :]
                    psO_tiles = [
                        psO.tile([P, QCH], f32, tag="psO", name=f"psO_{u}_{q}")
                        for q in range(NOB)
                    ]

                    def emit_O(pjt, ets, u=u, psO_tiles=psO_tiles):
                        for sw in range(NSW):
                            for q2 in range(SW // QCH):
                                q = sw * (SW // QCH) + q2
                                nc.tensor.matmul(
                                    psO_tiles[q],
                                    lhsT=v1[:, u, pjt, :],
                                    rhs=ets[sw][:, q2 * QCH : (q2 + 1) * QCH],
                                    start=(pjt == 0),
                                    stop=(pjt == NJT - 1),
                                )

                    prev = None
                    for jt in range(NJT):
                        ets = []
                        for sw in range(NSW):
                            ps = psS.tile([P, SW], f32, tag="psS")
                            for q2 in range(SW // QCH):
                                nc.tensor.matmul(
                                    ps[:, q2 * QCH : (q2 + 1) * QCH],
                                    lhsT=kT_u[:, jt * P : (jt + 1) * P],
                                    rhs=qT_u[
                                        :,
                                        sw * SW + q2 * QCH : sw * SW + (q2 + 1) * QCH,
                                    ],
                                    start=True,
                                    stop=True,
                                )
                            et = epool.tile([P, SW], f32r, tag="epool")
                            nc.scalar.activation(et, ps, EXP, scale=SCALE)
                            ets.append(et)
                        if prev is not None:
                            emit_O(jt - 1, prev)
                        prev = ets
                    emit_O(NJT - 1, prev)

                    # drain psO fast; normalize off the PSUM-release path
                    o_sb = obuf.tile([64, nseq], f32, tag="obuf", name=f"o_sb_{u}")
                    rs_sb = small.tile([1, nseq], f32, tag="rs")
                    for q in range(NOB):
                        nc.vector.tensor_copy(
                            o_sb[:, q * QCH : (q + 1) * QCH], psO_tiles[q][0:64, :]
                        )
                        nc.vector.tensor_copy(
                            rs_sb[:, q * QCH : (q + 1) * QCH],
                            psO_tiles[q][HD : HD + 1, :],
                        )
                    recip = small.tile([1, nseq], f32, tag="recip")
                    nc.vector.reciprocal_approx_fast(recip, rs_sb)
                    bcast = small.tile([64, nseq], f32, tag="bcast")
                    nc.gpsimd.partition_broadcast(bcast, recip)
                    for q in range(NOB):
                        nc.vector.tensor_mul(
                            OT[pb : pb + 64, u // 2, q * QCH : (q + 1) * QCH],
                            o_sb[:, q * QCH : (q + 1) * QCH],
                            bcast[:, q * QCH : (q + 1) * QCH],
                        )

            with (
                tc.tile_pool(name="opool", bufs=3) as opool,
                tc.tile_pool(name="psP", bufs=2, space="PSUM") as psP,
            ):
                # ---- proj partial out[i, e] ----
                for it in range(NIT):
                    for ech in range(C // ECH):
                        ps = psP.tile([P, ECH], f32, tag="psP")
                        for co in range(2):
                            nc.tensor.matmul(
                                ps,
                                lhsT=OT[:, co, it * P : (it + 1) * P],
                                rhs=wp_sb[:, co, ech * ECH : (ech + 1) * ECH],
                                start=(co == 0),
                                stop=(co == 1),
                            )
                        ot = opool.tile([P, ECH], f32, tag="opool")
                        cp(ot, ps)
                        nc.sync.dma_start(
                            out_d[it * P : (it + 1) * P, ech * ECH : (ech + 1) * ECH],
                            ot,
                        )

    nc.compile()
    return nc


def get_nc(nseq=NSEQ):
    if nseq not in _cache:
        _cache[nseq] = _build(nseq)
    return _cache[nseq]


def make_in_maps(x, w_qkv, w_proj, nseq=NSEQ):
    x = np.ascontiguousarray(x, dtype=np.float32)
    w_qkv = np.ascontiguousarray(w_qkv, dtype=np.float32)
    w_proj = np.ascontiguousarray(w_proj, dtype=np.float32)
    in_maps = []
    for core in range(8):
        b, hg = core // 4, core % 4
        hs = 4 * hg
        wsel = np.empty((6, P, C), np.float32)
        for mt in range(6):
            t, half = mt // 2, mt % 2
            r0 = t * C + (hs + 2 * half) * HD
            wsel[mt] = w_qkv[r0 : r0 + P, :]
        wqkvT = np.ascontiguousarray(wsel.transpose(2, 0, 1).reshape(C, 6 * P))
        wp = np.empty((P, 2, C), np.float32)
        for co in range(2):
            c0 = (hs + 2 * co) * HD
            wp[:, co, :] = w_proj[:, c0 : c0 + P].T
        in_maps.append(
            {"x": np.ascontiguousarray(x[b, :nseq]), "wqkvT": wqkvT, "wprojT": wp}
        )
    return in_maps


def kernel(x, w_qkv, w_proj, b_proj):
    from concourse.bass_utils import run_bass_kernel_spmd

    nc = get_nc()
    in_maps = make_in_maps(x, w_qkv, w_proj)
    res = run_bass_kernel_spmd(nc, in_maps, core_ids=list(range(8)))
    parts = [r["out"] for r in res.results]
    out = np.stack(
        [
            parts[0] + parts[1] + parts[2] + parts[3],
            parts[4] + parts[5] + parts[6] + parts[7],
        ],
        axis=0,
    )
    return (out + np.asarray(b_proj, np.float32)).astype(np.float32)
